# revision 5
# baseline (speedup 1.0000x reference)
"""Trainium2 Bass kernel for nn_KernelAxialMultiAttention (linear attention).

Math (per independent (b, m) slice; x: [T=256, C=512], N=8 heads, D=64):
  q = elu(x @ Wq.T) + 1          [T, C]   (heads along C)
  k = elu(x @ Wk.T) + 1
  ksum[c]   = sum_t k[t, c]
  krow[n,t] = sum_{c in head n} k[t, c]
  zden[n,t] = sum_{c in head n} q[t, c] * ksum[c];  z = 1/zden
  s[n, c]   = sum_t krow[n, t] * x[t, c]
  u[n, e]   = sum_c s[n, c] * Wv[n*D+e, c]     (= KtV column sums)
  w[n, cO]  = sum_e u[n, e] * Wp[cO, n*D+e]
  out[t,cO] = sum_n z[n, t] * w[n, cO]
Algebraically identical to the reference (sum reordering only); the
v-projection and output projection collapse because Z is constant over D.

elu(x)+1 = max(x,0) + exp(min(x,0)); computed per PSUM tile either as
  Q-variant: Scalar relu(-x), Scalar exp(-r), DVE (max(x,0) + e)
  K-variant: Scalar exp(x), DVE min(e,1), DVE (max(x,0) + c)
to balance Scalar vs DVE load.

All TensorEngine operands are bf16 (fp32 moving operand runs at 1/4 rate);
PSUM accumulation is fp32 everywhere.  The per-pair loop is software
pipelined: the projection matmuls of pair p are emitted before the small
reduction matmuls of pair p-1 so the PE never starves.

Sharding: data-parallel over the 128 (b, m) slices -> 16 per NeuronCore.
"""

import os
import sys

import numpy as np

for _p in ("/opt/trn_rl_repo", "/root/.axon_site/_ro/trn_rl_repo"):
    if os.path.isdir(_p) and _p not in sys.path:
        sys.path.insert(0, _p)

B, M, T, C = 2, 64, 256, 512
NH, D = 8, 64
S = 16            # slices per core
NCORES = 8
P = 128           # partitions
NKC = C // P      # 4 contraction chunks
NTC = T // P      # 2 t chunks

_BUILT = {}


def _build_nc():
    from contextlib import ExitStack

    import concourse.bacc as bacc
    import concourse.bass as bass
    import concourse.mybir as mybir
    import concourse.tile as tile
    from concourse.masks import make_identity

    f32 = mybir.dt.float32
    bf16 = mybir.dt.bfloat16
    AF = mybir.ActivationFunctionType
    OP = mybir.AluOpType
    AX = mybir.AxisListType

    nc = bacc.Bacc(None, target_bir_lowering=False)
    x_d = nc.declare_dram_parameter("x16", [S, T, C], bf16, isOutput=False)
    xT_d = nc.declare_dram_parameter("xT16", [S, C, T], bf16, isOutput=False)
    wqT_d = nc.declare_dram_parameter("WqT16", [C, C], bf16, isOutput=False)
    wkT_d = nc.declare_dram_parameter("WkT16", [C, C], bf16, isOutput=False)
    wvT_d = nc.declare_dram_parameter("WvT16", [C, C], bf16, isOutput=False)
    wpT_d = nc.declare_dram_parameter("WpT16", [C, C], bf16, isOutput=False)
    out_d = nc.declare_dram_parameter("out", [S, T, C], f32, isOutput=True)

    with tile.TileContext(nc) as tc, ExitStack() as ctx:
        wpool = ctx.enter_context(tc.tile_pool(name="weights", bufs=1))
        cpool = ctx.enter_context(tc.tile_pool(name="consts", bufs=1))
        persist = ctx.enter_context(tc.tile_pool(name="persist", bufs=1))
        xn_pool = ctx.enter_context(tc.tile_pool(name="xnat", bufs=4))
        xt_pool = ctx.enter_context(tc.tile_pool(name="xT", bufs=2))
        rn_pool = ctx.enter_context(tc.tile_pool(name="rneg", bufs=3))
        ex_pool = ctx.enter_context(tc.tile_pool(name="expt", bufs=3))
        ct_pool = ctx.enter_context(tc.tile_pool(name="clip", bufs=2))
        qe_pool = ctx.enter_context(tc.tile_pool(name="qe", bufs=2))
        ke_pool = ctx.enter_context(tc.tile_pool(name="ke", bufs=2))
        ksum_pool = ctx.enter_context(tc.tile_pool(name="ksum", bufs=2))
        krow_pool = ctx.enter_context(tc.tile_pool(name="krow", bufs=2))
        krt_pool = ctx.enter_context(tc.tile_pool(name="krowT", bufs=2))
        wz_pool = ctx.enter_context(tc.tile_pool(name="wz", bufs=2))
        gm_pool = ctx.enter_context(tc.tile_pool(name="gm", bufs=2))
        w4_pool = ctx.enter_context(tc.tile_pool(name="w4", bufs=2))
        zb_pool = ctx.enter_context(tc.tile_pool(name="zb", bufs=2))
        osb_pool = ctx.enter_context(tc.tile_pool(name="outsb", bufs=3))

        ps_proj = ctx.enter_context(
            tc.tile_pool(name="ps_proj", bufs=4, space=bass.MemorySpace.PSUM))
        ps_z = ctx.enter_context(
            tc.tile_pool(name="ps_z", bufs=1, space=bass.MemorySpace.PSUM))
        ps_tr = ctx.enter_context(
            tc.tile_pool(name="ps_tr", bufs=1, space=bass.MemorySpace.PSUM))
        ps_sm = ctx.enter_context(
            tc.tile_pool(name="ps_sm", bufs=2, space=bass.MemorySpace.PSUM))

        # ---- weights (host-pretransposed) into SBUF ----
        # layout [c % 128, c // 128, row]
        wqT = wpool.tile([P, NKC, C], bf16, tag="wqT")
        wkT = wpool.tile([P, NKC, C], bf16, tag="wkT")
        wvT = wpool.tile([P, NKC, C], bf16, tag="wvT")
        wpT = wpool.tile([P, NKC, C], bf16, tag="wpT")
        for wT, wd in ((wqT, wqT_d), (wkT, wkT_d), (wvT, wvT_d), (wpT, wpT_d)):
            nc.gpsimd.dma_start(
                out=wT[:], in_=wd.rearrange("(a p) d -> p a d", p=P))

        # ---- head-block masks: maskT[:, ci, n] = 1 if (128*ci + p)//64 == n ----
        maskT = cpool.tile([P, NKC, NH], bf16, tag="maskT")
        nc.gpsimd.memset(maskT[:], 0.0)
        for ci in range(NKC):
            nc.gpsimd.memset(maskT[0:64, ci, 2 * ci:2 * ci + 1], 1.0)
            nc.gpsimd.memset(maskT[64:128, ci, 2 * ci + 1:2 * ci + 2], 1.0)
        ident = cpool.tile([P, P], bf16, tag="ident")
        make_identity(nc, ident[:])

        sT_all = persist.tile([P, NKC, S, NH], bf16, tag="sT_all")
        z_all = persist.tile([P, S, T], f32, tag="z_all")
        uT_sb = persist.tile([P, NKC, S], f32, tag="uT_sb")

        x3 = x_d  # [S, T, C] bf16

        # ---------------- phase A helpers (software pipelined) --------------
        def emit_proj(p):
            s0, s1 = 2 * p, 2 * p + 1
            xn = []
            for s in (s0, s1):
                t_ = xn_pool.tile([P, NTC, C], bf16, tag="xnat")
                nc.sync.dma_start(
                    out=t_[:],
                    in_=x3[s].rearrange("(a p) c -> p a c", p=P),
                )
                xn.append(t_)
            xT = xt_pool.tile([P, NKC, 2, T], bf16, tag="xT")
            for si, s in ((0, s0), (1, s1)):
                nc.sync.dma_start(
                    out=xT[:, :, si, :],
                    in_=xT_d[s].rearrange("(a p) t -> p a t", p=P),
                )

            ksum = ksum_pool.tile([P, NKC, 2], f32, tag="ksum")
            qe = qe_pool.tile([P, NKC, 2 * T], bf16, tag="qe")
            ke = ke_pool.tile([P, NKC, 2 * T], bf16, tag="ke")
            for wT, etile, is_k in ((wqT, qe, False), (wkT, ke, True)):
                for mc in range(NKC):
                    pp = ps_proj.tile([P, 2 * T], f32, tag="proj")
                    for kc in range(NKC):
                        nc.tensor.matmul(
                            pp[:],
                            wT[:, kc, mc * P:(mc + 1) * P],
                            xT[:, kc, :, :],
                            start=(kc == 0),
                            stop=(kc == NKC - 1),
                        )
                    # elu(x)+1 = max(x,0) + exp(min(x,0))
                    if is_k and mc == 0:
                        # K-variant: exp on Scalar, min+combine on DVE
                        ex = ex_pool.tile([P, 2 * T], bf16, tag="expt")
                        nc.scalar.activation(ex[:], pp[:], AF.Exp)
                        ct = ct_pool.tile([P, 2 * T], bf16, tag="clip")
                        nc.vector.tensor_scalar_min(ct[:], ex[:], 1.0)
                        nc.vector.scalar_tensor_tensor(
                            etile[:, mc, :], pp[:], 0.0, ct[:], OP.max, OP.add)
                    else:
                        # Q-variant: relu(-x), exp(-r) on Scalar, combine on DVE
                        rn = rn_pool.tile([P, 2 * T], f32, tag="rneg")
                        nc.scalar.activation(rn[:], pp[:], AF.Relu, scale=-1.0)
                        ex = ex_pool.tile([P, 2 * T], bf16, tag="expt")
                        nc.scalar.activation(ex[:], rn[:], AF.Exp, scale=-1.0)
                        nc.vector.scalar_tensor_tensor(
                            etile[:, mc, :], pp[:], 0.0, ex[:], OP.max, OP.add)
                    if is_k:
                        for si in range(2):
                            nc.vector.tensor_reduce(
                                ksum[:, mc, si:si + 1],
                                etile[:, mc, si * T:(si + 1) * T],
                                AX.X, OP.add)
            return dict(p=p, s0=s0, s1=s1, xn=xn, qe=qe, ke=ke, ksum=ksum)

        def emit_tail(st):
            s0, s1, xn = st["s0"], st["s1"], st["xn"]
            qe, ke, ksum = st["qe"], st["ke"], st["ksum"]
            # krow[n, t2] = sum_c maskT[c, n] * ke[c, t2]   (t2 covers both slices)
            krow_ps = ps_proj.tile([P, 2 * T], f32, tag="proj")
            for mc in range(NKC):
                nc.tensor.matmul(
                    krow_ps[0:NH, :],
                    maskT[:, mc, :],
                    ke[:, mc, :],
                    start=(mc == 0),
                    stop=(mc == NKC - 1),
                )
            krow_sb = krow_pool.tile([P, 2 * T], bf16, tag="krow")
            nc.vector.tensor_copy(krow_sb[0:NH, :], krow_ps[0:NH, :])
            # transpose 128-col chunks: krt[t, j, n], j = 2*si + tcb
            krt_tr = ps_tr.tile([P, NKC, NH], bf16, tag="ktr")
            for j in range(4):
                nc.tensor.transpose(
                    krt_tr[:, j, :],
                    krow_sb[0:NH, j * P:(j + 1) * P],
                    ident[0:NH, 0:NH],
                )
            krt = krt_pool.tile([P, NKC, NH], bf16, tag="krt")
            nc.vector.tensor_copy(krt[:], krt_tr[:])

            zden_ps = ps_z.tile([P, 2, T], f32, tag="zden")
            for si, s in ((0, s0), (1, s1)):
                # sT[c, n] = sum_t x[t, c] * krowT[t, n]
                st_ps = ps_sm.tile([P, NKC, NH], f32, tag="st")
                for mc in range(NKC):
                    for tcb in range(NTC):
                        nc.tensor.matmul(
                            st_ps[:, mc, :],
                            xn[si][:, tcb, mc * P:(mc + 1) * P],
                            krt[:, 2 * si + tcb, :],
                            start=(tcb == 0),
                            stop=(tcb == NTC - 1),
                        )
                nc.vector.tensor_copy(sT_all[:, :, s, :], st_ps[:])

                # zden[n, t] = sum_c (maskT*ksum)[c, n] * qe[c, t]
                wz = wz_pool.tile([P, NKC, NH], bf16, tag="wz")
                for mc in range(NKC):
                    nc.gpsimd.tensor_scalar_mul(
                        wz[:, mc, :], maskT[:, mc, :], ksum[:, mc, si:si + 1])
                for mc in range(NKC):
                    nc.tensor.matmul(
                        zden_ps[0:NH, si, :],
                        wz[:, mc, :],
                        qe[:, mc, si * T:(si + 1) * T],
                        start=(mc == 0),
                        stop=(mc == NKC - 1),
                    )
            nc.vector.reciprocal_approx_fast(
                z_all[0:NH, s0:s0 + 2, :], zden_ps[0:NH, :, :])

        prev = None
        for p in range(S // 2):
            cur = emit_proj(p)
            if prev is not None:
                emit_tail(prev)
            prev = cur
        emit_tail(prev)

        # =================== phase B: u (batched over slices) ===============
        # uT[e, n, slice] = sum_c WvT[c, n*D+e] * sT[c, n, slice]
        ut_ps = ps_sm.tile([P, NKC, S], f32, tag="st")
        for n in range(NH):
            r0 = 64 * (n % 2)
            for kc in range(NKC):
                nc.tensor.matmul(
                    ut_ps[r0:r0 + 64, n // 2, :],
                    wvT[:, kc, n * D:(n + 1) * D],
                    sT_all[:, kc, :, n],
                    start=(kc == 0),
                    stop=(kc == NKC - 1),
                )
        nc.scalar.copy(uT_sb[:], ut_ps[:])

        # =================== phase C: w, out =================================
        zb = None
        for s in range(S):
            # GmaskT[c, ci, n] = maskT * uT  (block-diagonal masked u)
            gm = gm_pool.tile([P, NKC, NH], bf16, tag="gm")
            for ci in range(NKC):
                nc.gpsimd.tensor_scalar_mul(
                    gm[:, ci, :], maskT[:, ci, :], uT_sb[:, ci, s:s + 1])
            w4 = w4_pool.tile([P, C], bf16, tag="w4")
            w_ps = ps_proj.tile([P, C], f32, tag="proj")
            for ci in range(NKC):
                nc.tensor.matmul(
                    w_ps[0:NH, :],
                    gm[:, ci, :],
                    wpT[:, ci, :],
                    start=(ci == 0),
                    stop=(ci == NKC - 1),
                )
            nc.scalar.copy(w4[0:NH, :], w_ps[0:NH, :])

            if s % 2 == 0:
                zb = zb_pool.tile([P, 2, T], bf16, tag="zb")
                nc.vector.tensor_copy(zb[0:NH, :, :], z_all[0:NH, s:s + 2, :])

            osb = osb_pool.tile([P, NTC, C], f32, tag="outsb")
            for tcb in range(NTC):
                o_ps = ps_proj.tile([P, C], f32, tag="proj")
                nc.tensor.matmul(
                    o_ps[:],
                    zb[0:NH, s % 2, tcb * P:(tcb + 1) * P],
                    w4[0:NH, :],
                    start=True,
                    stop=True,
                )
                if tcb == 0:
                    nc.scalar.copy(osb[:, tcb, :], o_ps[:])
                else:
                    nc.vector.tensor_copy(osb[:, tcb, :], o_ps[:])
            nc.gpsimd.dma_start(
                out=out_d[s].rearrange("(a p) c -> p a c", p=P),
                in_=osb[:],
            )

    nc.compile()
    return nc


def _get_nc():
    if "nc" not in _BUILT:
        _BUILT["nc"] = _build_nc()
    return _BUILT["nc"]


def kernel(**inputs):
    import ml_dtypes

    bf16 = ml_dtypes.bfloat16
    x = np.asarray(inputs["x"], dtype=np.float32)
    Wq = np.asarray(inputs["Wq"], dtype=np.float32)
    Wk = np.asarray(inputs["Wk"], dtype=np.float32)
    Wv = np.asarray(inputs["Wv"], dtype=np.float32)
    Wp = np.asarray(inputs["Wp"], dtype=np.float32)
    bp = np.asarray(inputs.get("bp", np.zeros(C)), dtype=np.float32)

    x16 = np.ascontiguousarray(x.reshape(B * M, T, C).astype(bf16))
    xT16 = np.ascontiguousarray(x16.transpose(0, 2, 1))
    wqT16 = np.ascontiguousarray(Wq.T.astype(bf16))
    wkT16 = np.ascontiguousarray(Wk.T.astype(bf16))
    wvT16 = np.ascontiguousarray(Wv.T.astype(bf16))
    wpT16 = np.ascontiguousarray(Wp.T.astype(bf16))
    in_maps = []
    for i in range(NCORES):
        in_maps.append({
            "x16": np.ascontiguousarray(x16[S * i:S * (i + 1)]),
            "xT16": np.ascontiguousarray(xT16[S * i:S * (i + 1)]),
            "WqT16": wqT16, "WkT16": wkT16, "WvT16": wvT16, "WpT16": wpT16,
        })

    from concourse.bass_utils import run_bass_kernel_spmd

    nc = _get_nc()
    trace = os.environ.get("KERNEL_TRACE", "0") == "1"
    tdir = os.environ.get("KERNEL_TRACE_DIR") or None
    res = run_bass_kernel_spmd(nc, in_maps, list(range(NCORES)), trace=trace,
                               tmpdir=tdir)
    if trace and res.exec_time_ns is not None:
        print(f"HW exec time: {res.exec_time_ns} ns", flush=True)
        _BUILT["exec_time_ns"] = res.exec_time_ns
    if trace and res.instructions_and_trace is not None:
        _BUILT["trace_path"] = res.instructions_and_trace[1]

    out = np.concatenate([res.results[i]["out"] for i in range(NCORES)], axis=0)
    out = out.reshape(B, M, T, C)
    if np.any(bp):
        out = out + bp
    return out.astype(np.float32)


# revision 7
# speedup vs baseline: 1.0963x; 1.0963x over previous
"""Trainium2 Bass kernel for nn_KernelAxialMultiAttention (linear attention).

Math (per independent (b, m) slice; x: [T=256, C=512], N=8 heads, D=64):
  q = elu(x @ Wq.T) + 1          [T, C]   (heads along C)
  k = elu(x @ Wk.T) + 1
  ksum[c]   = sum_t k[t, c]
  krow[n,t] = sum_{c in head n} k[t, c]
  zden[n,t] = sum_{c in head n} q[t, c] * ksum[c];  z = 1/zden
  s[n, c]   = sum_t krow[n, t] * x[t, c]
  u[n, e]   = sum_c s[n, c] * Wv[n*D+e, c]     (= KtV column sums)
  w[n, cO]  = sum_e u[n, e] * Wp[cO, n*D+e]
  out[t,cO] = sum_n z[n, t] * w[n, cO]
Algebraically identical to the reference (sum reordering only); the
v-projection and output projection collapse because Z is constant over D.

elu(x)+1 = max(x,0) + exp(min(x,0)); computed per PSUM tile either as
  Q-variant: Scalar relu(-x), Scalar exp(-r), DVE (max(x,0) + e)
  K-variant: Scalar exp(x), DVE min(e,1), DVE (max(x,0) + c)
to balance Scalar vs DVE load.

All TensorEngine operands are bf16 (fp32 moving operand runs at 1/4 rate);
PSUM accumulation is fp32 everywhere.  The per-pair loop is software
pipelined: the projection matmuls of pair p are emitted before the small
reduction matmuls of pair p-1 so the PE never starves.

Sharding: data-parallel over the 128 (b, m) slices -> 16 per NeuronCore.
"""

import os
import sys

import numpy as np

for _p in ("/opt/trn_rl_repo", "/root/.axon_site/_ro/trn_rl_repo"):
    if os.path.isdir(_p) and _p not in sys.path:
        sys.path.insert(0, _p)

B, M, T, C = 2, 64, 256, 512
NH, D = 8, 64
S = 16            # slices per core
NCORES = 8
P = 128           # partitions
NKC = C // P      # 4 contraction chunks
NTC = T // P      # 2 t chunks

_BUILT = {}


def _build_nc():
    from contextlib import ExitStack

    import concourse.bacc as bacc
    import concourse.bass as bass
    import concourse.mybir as mybir
    import concourse.tile as tile
    from concourse.masks import make_identity

    f32 = mybir.dt.float32
    bf16 = mybir.dt.bfloat16
    AF = mybir.ActivationFunctionType
    OP = mybir.AluOpType
    AX = mybir.AxisListType

    nc = bacc.Bacc(None, target_bir_lowering=False)
    x_d = nc.declare_dram_parameter("x16", [S, T, C], bf16, isOutput=False)
    xT_d = nc.declare_dram_parameter("xT16", [S, C, T], bf16, isOutput=False)
    wqT_d = nc.declare_dram_parameter("WqT16", [C, C], bf16, isOutput=False)
    wkT_d = nc.declare_dram_parameter("WkT16", [C, C], bf16, isOutput=False)
    wvT_d = nc.declare_dram_parameter("WvT16", [C, C], bf16, isOutput=False)
    wpT_d = nc.declare_dram_parameter("WpT16", [C, C], bf16, isOutput=False)
    out_d = nc.declare_dram_parameter("out", [S, T, C], f32, isOutput=True)
    wtmp_d = nc.declare_dram_parameter("wtmp", [S * NH, C], bf16, isOutput=True)

    with tile.TileContext(nc) as tc, ExitStack() as ctx:
        wpool = ctx.enter_context(tc.tile_pool(name="weights", bufs=1))
        cpool = ctx.enter_context(tc.tile_pool(name="consts", bufs=1))
        persist = ctx.enter_context(tc.tile_pool(name="persist", bufs=1))
        xn_pool = ctx.enter_context(tc.tile_pool(name="xnat", bufs=4))
        xt_pool = ctx.enter_context(tc.tile_pool(name="xT", bufs=2))
        rn_pool = ctx.enter_context(tc.tile_pool(name="rneg", bufs=3))
        ex_pool = ctx.enter_context(tc.tile_pool(name="expt", bufs=3))
        ct_pool = ctx.enter_context(tc.tile_pool(name="clip", bufs=2))
        qe_pool = ctx.enter_context(tc.tile_pool(name="qe", bufs=2))
        ke_pool = ctx.enter_context(tc.tile_pool(name="ke", bufs=2))
        ksum_pool = ctx.enter_context(tc.tile_pool(name="ksum", bufs=2))
        krow_pool = ctx.enter_context(tc.tile_pool(name="krow", bufs=2))
        krt_pool = ctx.enter_context(tc.tile_pool(name="krowT", bufs=2))
        wz_pool = ctx.enter_context(tc.tile_pool(name="wz", bufs=2))
        zb_pool = ctx.enter_context(tc.tile_pool(name="zb", bufs=8))
        osb_pool = ctx.enter_context(tc.tile_pool(name="outsb", bufs=3))

        ps_proj = ctx.enter_context(
            tc.tile_pool(name="ps_proj", bufs=4, space=bass.MemorySpace.PSUM))
        ps_z = ctx.enter_context(
            tc.tile_pool(name="ps_z", bufs=1, space=bass.MemorySpace.PSUM))
        ps_tr = ctx.enter_context(
            tc.tile_pool(name="ps_tr", bufs=1, space=bass.MemorySpace.PSUM))
        ps_sm = ctx.enter_context(
            tc.tile_pool(name="ps_sm", bufs=1, space=bass.MemorySpace.PSUM))
        ps_kr = ctx.enter_context(
            tc.tile_pool(name="ps_kr", bufs=1, space=bass.MemorySpace.PSUM))

        # ---- weights (host-pretransposed) into SBUF ----
        # layout [c % 128, c // 128, row]
        wqT = wpool.tile([P, NKC, C], bf16, tag="wqT")
        wkT = wpool.tile([P, NKC, C], bf16, tag="wkT")
        wvT = wpool.tile([P, NKC, C], bf16, tag="wvT")
        wpT = wpool.tile([P, NKC, C], bf16, tag="wpT")
        for wT, wd in ((wqT, wqT_d), (wkT, wkT_d), (wvT, wvT_d), (wpT, wpT_d)):
            nc.gpsimd.dma_start(
                out=wT[:], in_=wd.rearrange("(a p) d -> p a d", p=P))

        # ---- head-block masks: maskT[:, ci, n] = 1 if (128*ci + p)//64 == n ----
        maskT = cpool.tile([P, NKC, NH], bf16, tag="maskT")
        nc.gpsimd.memset(maskT[:], 0.0)
        for ci in range(NKC):
            nc.gpsimd.memset(maskT[0:64, ci, 2 * ci:2 * ci + 1], 1.0)
            nc.gpsimd.memset(maskT[64:128, ci, 2 * ci + 1:2 * ci + 2], 1.0)
        ident = cpool.tile([P, P], bf16, tag="ident")
        make_identity(nc, ident[:])

        sT_all = persist.tile([P, NKC, S, NH], bf16, tag="sT_all")
        z_all = persist.tile([P, S, T], f32, tag="z_all")
        uT_sb = persist.tile([P, NKC, S], f32, tag="uT_sb")

        x3 = x_d  # [S, T, C] bf16

        # ---------------- phase A helpers (software pipelined) --------------
        def emit_proj(p):
            s0, s1 = 2 * p, 2 * p + 1
            xn = []
            for s in (s0, s1):
                t_ = xn_pool.tile([P, NTC, C], bf16, tag="xnat")
                nc.sync.dma_start(
                    out=t_[:],
                    in_=x3[s].rearrange("(a p) c -> p a c", p=P),
                )
                xn.append(t_)
            xT = xt_pool.tile([P, NKC, 2, T], bf16, tag="xT")
            for si, s in ((0, s0), (1, s1)):
                nc.sync.dma_start(
                    out=xT[:, :, si, :],
                    in_=xT_d[s].rearrange("(a p) t -> p a t", p=P),
                )

            ksum = ksum_pool.tile([P, NKC, 2], f32, tag="ksum")
            qe = qe_pool.tile([P, NKC, 2 * T], bf16, tag="qe")
            ke = ke_pool.tile([P, NKC, 2 * T], bf16, tag="ke")
            for wT, etile, is_k in ((wqT, qe, False), (wkT, ke, True)):
                for mc in range(NKC):
                    pp = ps_proj.tile([P, 2 * T], f32, tag="proj")
                    for kc in range(NKC):
                        nc.tensor.matmul(
                            pp[:],
                            wT[:, kc, mc * P:(mc + 1) * P],
                            xT[:, kc, :, :],
                            start=(kc == 0),
                            stop=(kc == NKC - 1),
                        )
                    # elu(x)+1 = max(x,0) + exp(min(x,0))
                    if is_k and mc == 0:
                        # K-variant: exp on Scalar, min+combine on DVE
                        ex = ex_pool.tile([P, 2 * T], bf16, tag="expt")
                        nc.scalar.activation(ex[:], pp[:], AF.Exp)
                        ct = ct_pool.tile([P, 2 * T], bf16, tag="clip")
                        nc.vector.tensor_scalar_min(ct[:], ex[:], 1.0)
                        nc.vector.scalar_tensor_tensor(
                            etile[:, mc, :], pp[:], 0.0, ct[:], OP.max, OP.add)
                    else:
                        # Q-variant: relu(-x), exp(-r) on Scalar, combine on DVE
                        rn = rn_pool.tile([P, 2 * T], f32, tag="rneg")
                        nc.scalar.activation(rn[:], pp[:], AF.Relu, scale=-1.0)
                        ex = ex_pool.tile([P, 2 * T], bf16, tag="expt")
                        nc.scalar.activation(ex[:], rn[:], AF.Exp, scale=-1.0)
                        nc.vector.scalar_tensor_tensor(
                            etile[:, mc, :], pp[:], 0.0, ex[:], OP.max, OP.add)
                    if is_k:
                        nc.vector.tensor_reduce(
                            ksum[:, mc, :],
                            etile[:, mc, :].rearrange("p (a t) -> p a t", a=2),
                            AX.X, OP.add)
            return dict(p=p, s0=s0, s1=s1, xn=xn, qe=qe, ke=ke, ksum=ksum)

        def emit_tail(st):
            s0, s1, xn = st["s0"], st["s1"], st["xn"]
            qe, ke, ksum = st["qe"], st["ke"], st["ksum"]
            # krow[n, t2] = sum_c maskT[c, n] * ke[c, t2]   (t2 covers both slices)
            krow_ps = ps_kr.tile([P, 2 * T], f32, tag="krow")
            for mc in range(NKC):
                nc.tensor.matmul(
                    krow_ps[0:NH, :],
                    maskT[:, mc, :],
                    ke[:, mc, :],
                    start=(mc == 0),
                    stop=(mc == NKC - 1),
                )
            krow_sb = krow_pool.tile([P, 2 * T], bf16, tag="krow")
            nc.vector.tensor_copy(krow_sb[0:NH, :], krow_ps[0:NH, :])
            # transpose 128-col chunks: krt[t, j, n], j = 2*si + tcb
            krt_tr = ps_tr.tile([P, NKC, NH], bf16, tag="ktr")
            for j in range(4):
                nc.tensor.transpose(
                    krt_tr[:, j, :],
                    krow_sb[0:NH, j * P:(j + 1) * P],
                    ident[0:NH, 0:NH],
                )
            krt = krt_pool.tile([P, NKC, NH], bf16, tag="krt")
            nc.vector.tensor_copy(krt[:], krt_tr[:])

            zden_ps = ps_z.tile([P, 2, T], f32, tag="zden")
            for si, s in ((0, s0), (1, s1)):
                # sT[c, n] = sum_t x[t, c] * krowT[t, n]
                st_ps = ps_sm.tile([P, NKC, NH], f32, tag="st")
                for mc in range(NKC):
                    for tcb in range(NTC):
                        nc.tensor.matmul(
                            st_ps[:, mc, :],
                            xn[si][:, tcb, mc * P:(mc + 1) * P],
                            krt[:, 2 * si + tcb, :],
                            start=(tcb == 0),
                            stop=(tcb == NTC - 1),
                        )
                nc.vector.tensor_copy(sT_all[:, :, s, :], st_ps[:])

                # zden[n, t] = sum_c (maskT*ksum)[c, n] * qe[c, t]
                wz = wz_pool.tile([P, NKC, NH], bf16, tag="wz")
                for mc in range(NKC):
                    nc.gpsimd.tensor_scalar_mul(
                        wz[:, mc, :], maskT[:, mc, :], ksum[:, mc, si:si + 1])
                for mc in range(NKC):
                    nc.tensor.matmul(
                        zden_ps[0:NH, si, :],
                        wz[:, mc, :],
                        qe[:, mc, si * T:(si + 1) * T],
                        start=(mc == 0),
                        stop=(mc == NKC - 1),
                    )
            nc.vector.reciprocal_approx_fast(
                z_all[0:NH, s0:s0 + 2, :], zden_ps[0:NH, :, :])

        prev = None
        for p in range(S // 2):
            cur = emit_proj(p)
            if prev is not None:
                emit_tail(prev)
            prev = cur
        emit_tail(prev)

        # =================== phase B: u (batched over slices) ===============
        # uT[e, n, slice] = sum_c WvT[c, n*D+e] * sT[c, n, slice]
        ut_ps = ps_sm.tile([P, NKC, S], f32, tag="st")
        for n in range(NH):
            r0 = 64 * (n % 2)
            for kc in range(NKC):
                nc.tensor.matmul(
                    ut_ps[r0:r0 + 64, n // 2, :],
                    wvT[:, kc, n * D:(n + 1) * D],
                    sT_all[:, kc, :, n],
                    start=(kc == 0),
                    stop=(kc == NKC - 1),
                )
        nc.scalar.copy(uT_sb[:], ut_ps[:])

        # cast z to bf16 for the final matmuls (overlaps the GM build below)
        zbs = []
        for pr in range(S // 2):
            zb = zb_pool.tile([P, 2, T], bf16, tag="zb")
            nc.vector.tensor_copy(zb[0:NH, :, :], z_all[0:NH, 2 * pr:2 * pr + 2, :])
            zbs.append(zb)

        # ============ phase C: batched w, then per-slice out ================
        # GM_all[c, ci, 8*s + n] = maskT[c, ci, n] * uT[c, ci, s]
        GM_all = persist.tile([P, NKC, S * NH], bf16, tag="GM")
        engs = (nc.scalar, nc.gpsimd, nc.vector)
        idx = 0
        for ci in range(NKC):
            for s in range(S):
                eng = engs[idx % 3]
                idx += 1
                if eng is nc.scalar:
                    nc.scalar.mul(
                        GM_all[:, ci, 8 * s:8 * s + 8],
                        maskT[:, ci, :], uT_sb[:, ci, s:s + 1])
                else:
                    eng.tensor_scalar_mul(
                        GM_all[:, ci, 8 * s:8 * s + 8],
                        maskT[:, ci, :], uT_sb[:, ci, s:s + 1])

        # W_ps[8*s + n, cO] = sum_c GM_all[c, (8s+n)] * WpT[c, cO]
        w_ps = ps_proj.tile([P, C], f32, tag="proj")
        for ci in range(NKC):
            nc.tensor.matmul(
                w_ps[:],
                GM_all[:, ci, :],
                wpT[:, ci, :],
                start=(ci == 0),
                stop=(ci == NKC - 1),
            )
        w4sb = persist.tile([P, C], bf16, tag="w4sb")
        nc.scalar.copy(w4sb[:], w_ps[:])
        # shuffle rows (8s+n) -> partition n, free s via a DRAM round-trip
        # (same DGE queue => ordered)
        nc.scalar.dma_start(out=wtmp_d[:, :], in_=w4sb[:])
        w4stk = persist.tile([P, S, C], bf16, tag="w4stk")
        nc.scalar.dma_start(
            out=w4stk[0:NH, :, :],
            in_=wtmp_d.rearrange("(s n) c -> n s c", n=NH),
        )

        for s in range(S):
            osb = osb_pool.tile([P, NTC, C], f32, tag="outsb")
            for tcb in range(NTC):
                o_ps = ps_proj.tile([P, C], f32, tag="proj")
                nc.tensor.matmul(
                    o_ps[:],
                    zbs[s // 2][0:NH, s % 2, tcb * P:(tcb + 1) * P],
                    w4stk[0:NH, s, :],
                    start=True,
                    stop=True,
                )
                if tcb == 0:
                    nc.scalar.copy(osb[:, tcb, :], o_ps[:])
                else:
                    nc.vector.tensor_copy(osb[:, tcb, :], o_ps[:])
            nc.gpsimd.dma_start(
                out=out_d[s].rearrange("(a p) c -> p a c", p=P),
                in_=osb[:],
            )

    nc.compile()
    return nc


def _get_nc():
    if "nc" not in _BUILT:
        _BUILT["nc"] = _build_nc()
    return _BUILT["nc"]


def kernel(**inputs):
    import ml_dtypes

    bf16 = ml_dtypes.bfloat16
    x = np.asarray(inputs["x"], dtype=np.float32)
    Wq = np.asarray(inputs["Wq"], dtype=np.float32)
    Wk = np.asarray(inputs["Wk"], dtype=np.float32)
    Wv = np.asarray(inputs["Wv"], dtype=np.float32)
    Wp = np.asarray(inputs["Wp"], dtype=np.float32)
    bp = np.asarray(inputs.get("bp", np.zeros(C)), dtype=np.float32)

    x16 = np.ascontiguousarray(x.reshape(B * M, T, C).astype(bf16))
    xT16 = np.ascontiguousarray(x16.transpose(0, 2, 1))
    wqT16 = np.ascontiguousarray(Wq.T.astype(bf16))
    wkT16 = np.ascontiguousarray(Wk.T.astype(bf16))
    wvT16 = np.ascontiguousarray(Wv.T.astype(bf16))
    wpT16 = np.ascontiguousarray(Wp.T.astype(bf16))
    in_maps = []
    for i in range(NCORES):
        in_maps.append({
            "x16": np.ascontiguousarray(x16[S * i:S * (i + 1)]),
            "xT16": np.ascontiguousarray(xT16[S * i:S * (i + 1)]),
            "WqT16": wqT16, "WkT16": wkT16, "WvT16": wvT16, "WpT16": wpT16,
        })

    from concourse.bass_utils import run_bass_kernel_spmd

    nc = _get_nc()
    trace = os.environ.get("KERNEL_TRACE", "0") == "1"
    tdir = os.environ.get("KERNEL_TRACE_DIR") or None
    res = run_bass_kernel_spmd(nc, in_maps, list(range(NCORES)), trace=trace,
                               tmpdir=tdir)
    if trace and res.exec_time_ns is not None:
        print(f"HW exec time: {res.exec_time_ns} ns", flush=True)
        _BUILT["exec_time_ns"] = res.exec_time_ns
    if trace and res.instructions_and_trace is not None:
        _BUILT["trace_path"] = res.instructions_and_trace[1]

    out = np.concatenate([res.results[i]["out"] for i in range(NCORES)], axis=0)
    out = out.reshape(B, M, T, C)
    if np.any(bp):
        out = out + bp
    return out.astype(np.float32)


# revision 9
# speedup vs baseline: 1.1225x; 1.0239x over previous
"""Trainium2 Bass kernel for nn_KernelAxialMultiAttention (linear attention).

Math (per independent (b, m) slice; x: [T=256, C=512], N=8 heads, D=64):
  q = elu(x @ Wq.T) + 1          [T, C]   (heads along C)
  k = elu(x @ Wk.T) + 1
  ksum[c]   = sum_t k[t, c]
  krow[n,t] = sum_{c in head n} k[t, c]
  zden[n,t] = sum_{c in head n} q[t, c] * ksum[c];  z = 1/zden
  s[n, c]   = sum_t krow[n, t] * x[t, c]
  u[n, e]   = sum_c s[n, c] * Wv[n*D+e, c]     (= KtV column sums)
  w[n, cO]  = sum_e u[n, e] * Wp[cO, n*D+e]
  out[t,cO] = sum_n z[n, t] * w[n, cO]
Algebraically identical to the reference (sum reordering only); the
v-projection and output projection collapse because Z is constant over D.

elu(x)+1 = max(x,0) + exp(min(x,0)); computed per PSUM tile either as
  Q-variant: Scalar relu(-x), Scalar exp(-r), DVE (max(x,0) + e)
  K-variant: Scalar exp(x), DVE min(e,1), DVE (max(x,0) + c)
to balance Scalar vs DVE load.

All TensorEngine operands are bf16 (fp32 moving operand runs at 1/4 rate);
PSUM accumulation is fp32 everywhere.  The per-pair loop is software
pipelined: the projection matmuls of pair p are emitted before the small
reduction matmuls of pair p-1 so the PE never starves.

Sharding: data-parallel over the 128 (b, m) slices -> 16 per NeuronCore.
"""

import os
import sys

import numpy as np

for _p in ("/opt/trn_rl_repo", "/root/.axon_site/_ro/trn_rl_repo"):
    if os.path.isdir(_p) and _p not in sys.path:
        sys.path.insert(0, _p)

B, M, T, C = 2, 64, 256, 512
NH, D = 8, 64
S = 16            # slices per core
NCORES = 8
P = 128           # partitions
NKC = C // P      # 4 contraction chunks
NTC = T // P      # 2 t chunks

_BUILT = {}


def _register_elu_op():
    """Register a fused custom-DVE op: out = max(in0, 0) + min(in1, s0).

    This is the documented extension point for custom DVE ops
    (concourse/dve_ops.py docstring); we register at runtime since the
    repo tree is read-only here."""
    import concourse.dve_ops as dve_ops
    for op in dve_ops.OPS:
        if op.name == "ELU1_COMBINE_ANT":
            return op
    from concourse.dve_spec import (
        C0, Spec, Src0, Src1, Zero, _has_src1, lower, maxx, minn,
    )
    from concourse.dve_uop import DveOpSpec

    name = "ELU1_COMBINE_ANT"
    row = dve_ops._CUSTOM_DVE_ROW_BASE + len(dve_ops.OPS)
    assert row < 0x20
    dve_ops._SUB_OPCODE_FOR_NAME[name] = row
    spec = Spec(
        body=maxx(Src0, Zero) + minn(Src1, C0),
        reference=lambda in0, in1, s0, s1, imm2: (
            np.maximum(in0, 0.0) + np.minimum(in1, s0)
        ).astype(np.float32),
    )
    shas = {}
    for ver in ("v3", "v4"):
        try:
            uops = lower(spec, ver=ver)
            shas[ver] = DveOpSpec(
                name=name, opcode=row, uops=uops, rd1_en=_has_src1(spec)
            ).sha(ver)
        except Exception:
            pass
    op = dve_ops.DveOp(name, spec, subdim=False, uops_sha=shas)
    dve_ops.OPS.append(op)
    dve_ops.CUSTOM_DVE_SPECS[name] = spec
    return op


def _build_nc():
    from contextlib import ExitStack

    import concourse.bacc as bacc
    import concourse.bass as bass
    import concourse.mybir as mybir
    import concourse.tile as tile
    from concourse.masks import make_identity

    f32 = mybir.dt.float32
    bf16 = mybir.dt.bfloat16
    AF = mybir.ActivationFunctionType
    OP = mybir.AluOpType
    AX = mybir.AxisListType

    elu_op = _register_elu_op()

    nc = bacc.Bacc(None, target_bir_lowering=False)
    x_d = nc.declare_dram_parameter("x16", [S, T, C], bf16, isOutput=False)
    xT_d = nc.declare_dram_parameter("xT16", [S, C, T], bf16, isOutput=False)
    wqT_d = nc.declare_dram_parameter("WqT16", [C, C], bf16, isOutput=False)
    wkT_d = nc.declare_dram_parameter("WkT16", [C, C], bf16, isOutput=False)
    wvT_d = nc.declare_dram_parameter("WvT16", [C, C], bf16, isOutput=False)
    wpT_d = nc.declare_dram_parameter("WpT16", [C, C], bf16, isOutput=False)
    out_d = nc.declare_dram_parameter("out", [S, T, C], f32, isOutput=True)
    wtmp_d = nc.declare_dram_parameter("wtmp", [S * NH, C], bf16, isOutput=True)

    with tile.TileContext(nc) as tc, ExitStack() as ctx:
        wpool = ctx.enter_context(tc.tile_pool(name="weights", bufs=1))
        cpool = ctx.enter_context(tc.tile_pool(name="consts", bufs=1))
        persist = ctx.enter_context(tc.tile_pool(name="persist", bufs=1))
        xn_pool = ctx.enter_context(tc.tile_pool(name="xnat", bufs=4))
        xt_pool = ctx.enter_context(tc.tile_pool(name="xT", bufs=2))
        ex_pool = ctx.enter_context(tc.tile_pool(name="expt", bufs=3))
        qe_pool = ctx.enter_context(tc.tile_pool(name="qe", bufs=2))
        ke_pool = ctx.enter_context(tc.tile_pool(name="ke", bufs=2))
        ksum_pool = ctx.enter_context(tc.tile_pool(name="ksum", bufs=2))
        krow_pool = ctx.enter_context(tc.tile_pool(name="krow", bufs=2))
        krt_pool = ctx.enter_context(tc.tile_pool(name="krowT", bufs=2))
        wz_pool = ctx.enter_context(tc.tile_pool(name="wz", bufs=2))
        zb_pool = ctx.enter_context(tc.tile_pool(name="zb", bufs=8))
        osb_pool = ctx.enter_context(tc.tile_pool(name="outsb", bufs=3))

        ps_proj = ctx.enter_context(
            tc.tile_pool(name="ps_proj", bufs=4, space=bass.MemorySpace.PSUM))
        ps_z = ctx.enter_context(
            tc.tile_pool(name="ps_z", bufs=1, space=bass.MemorySpace.PSUM))
        ps_tr = ctx.enter_context(
            tc.tile_pool(name="ps_tr", bufs=1, space=bass.MemorySpace.PSUM))
        ps_sm = ctx.enter_context(
            tc.tile_pool(name="ps_sm", bufs=1, space=bass.MemorySpace.PSUM))
        ps_kr = ctx.enter_context(
            tc.tile_pool(name="ps_kr", bufs=1, space=bass.MemorySpace.PSUM))

        # ---- weights (host-pretransposed) into SBUF ----
        # layout [c % 128, c // 128, row]
        wqT = wpool.tile([P, NKC, C], bf16, tag="wqT")
        wkT = wpool.tile([P, NKC, C], bf16, tag="wkT")
        wvT = wpool.tile([P, NKC, C], bf16, tag="wvT")
        wpT = wpool.tile([P, NKC, C], bf16, tag="wpT")
        for wT, wd in ((wqT, wqT_d), (wkT, wkT_d)):
            nc.gpsimd.dma_start(
                out=wT[:], in_=wd.rearrange("(a p) d -> p a d", p=P))

        # ---- head-block masks: maskT[:, ci, n] = 1 if (128*ci + p)//64 == n ----
        maskT = cpool.tile([P, NKC, NH], bf16, tag="maskT")
        nc.gpsimd.memset(maskT[:], 0.0)
        for ci in range(NKC):
            nc.gpsimd.memset(maskT[0:64, ci, 2 * ci:2 * ci + 1], 1.0)
            nc.gpsimd.memset(maskT[64:128, ci, 2 * ci + 1:2 * ci + 2], 1.0)
        ident = cpool.tile([P, P], bf16, tag="ident")
        make_identity(nc, ident[:])

        sT_all = persist.tile([P, NKC, S, NH], bf16, tag="sT_all")
        z_all = persist.tile([P, S, T], f32, tag="z_all")
        uT_sb = persist.tile([P, NKC, S], f32, tag="uT_sb")

        x3 = x_d  # [S, T, C] bf16

        # ---------------- phase A helpers (software pipelined) --------------
        def emit_proj(p):
            s0, s1 = 2 * p, 2 * p + 1
            xn = []
            for s in (s0, s1):
                t_ = xn_pool.tile([P, NTC, C], bf16, tag="xnat")
                nc.sync.dma_start(
                    out=t_[:],
                    in_=x3[s].rearrange("(a p) c -> p a c", p=P),
                )
                xn.append(t_)
            xT = xt_pool.tile([P, NKC, 2, T], bf16, tag="xT")
            for si, s in ((0, s0), (1, s1)):
                nc.sync.dma_start(
                    out=xT[:, :, si, :],
                    in_=xT_d[s].rearrange("(a p) t -> p a t", p=P),
                )

            ksum = ksum_pool.tile([P, NKC, 2], f32, tag="ksum")
            qe = qe_pool.tile([P, NKC, 2 * T], bf16, tag="qe")
            ke = ke_pool.tile([P, NKC, 2 * T], bf16, tag="ke")
            for wT, etile, is_k in ((wqT, qe, False), (wkT, ke, True)):
                for mc in range(NKC):
                    pp = ps_proj.tile([P, 2 * T], f32, tag="proj")
                    for kc in range(NKC):
                        nc.tensor.matmul(
                            pp[:],
                            wT[:, kc, mc * P:(mc + 1) * P],
                            xT[:, kc, :, :],
                            start=(kc == 0),
                            stop=(kc == NKC - 1),
                        )
                    # elu(x)+1 = max(x,0) + min(exp(x),1): Scalar exp,
                    # then one fused custom-DVE combine.
                    ex = ex_pool.tile([P, 2 * T], bf16, tag="expt")
                    nc.scalar.activation(ex[:], pp[:], AF.Exp)
                    nc.vector._custom_dve(
                        elu_op, out=etile[:, mc, :], in0=pp[:], in1=ex[:],
                        s0=1.0)
                    if is_k:
                        nc.vector.tensor_reduce(
                            ksum[:, mc, :],
                            etile[:, mc, :].rearrange("p (a t) -> p a t", a=2),
                            AX.X, OP.add)
            return dict(p=p, s0=s0, s1=s1, xn=xn, qe=qe, ke=ke, ksum=ksum)

        def emit_tail(st):
            s0, s1, xn = st["s0"], st["s1"], st["xn"]
            qe, ke, ksum = st["qe"], st["ke"], st["ksum"]
            # krow[n, t2] = sum_c maskT[c, n] * ke[c, t2]   (t2 covers both slices)
            krow_ps = ps_kr.tile([P, 2 * T], f32, tag="krow")
            for mc in range(NKC):
                nc.tensor.matmul(
                    krow_ps[0:NH, :],
                    maskT[:, mc, :],
                    ke[:, mc, :],
                    start=(mc == 0),
                    stop=(mc == NKC - 1),
                )
            krow_sb = krow_pool.tile([P, 2 * T], bf16, tag="krow")
            nc.scalar.copy(krow_sb[0:NH, :], krow_ps[0:NH, :])
            # transpose 128-col chunks: krt[t, j, n], j = 2*si + tcb
            krt_tr = ps_tr.tile([P, NKC, NH], bf16, tag="ktr")
            for j in range(4):
                nc.tensor.transpose(
                    krt_tr[:, j, :],
                    krow_sb[0:NH, j * P:(j + 1) * P],
                    ident[0:NH, 0:NH],
                )
            krt = krt_pool.tile([P, NKC, NH], bf16, tag="krt")
            nc.vector.tensor_copy(krt[:], krt_tr[:])

            zden_ps = ps_z.tile([P, 2, T], f32, tag="zden")
            for si, s in ((0, s0), (1, s1)):
                # sT[c, n] = sum_t x[t, c] * krowT[t, n]
                st_ps = ps_sm.tile([P, NKC, NH], f32, tag="st")
                for mc in range(NKC):
                    for tcb in range(NTC):
                        nc.tensor.matmul(
                            st_ps[:, mc, :],
                            xn[si][:, tcb, mc * P:(mc + 1) * P],
                            krt[:, 2 * si + tcb, :],
                            start=(tcb == 0),
                            stop=(tcb == NTC - 1),
                        )
                nc.vector.tensor_copy(sT_all[:, :, s, :], st_ps[:])

                # zden[n, t] = sum_c (maskT*ksum)[c, n] * qe[c, t]
                wz = wz_pool.tile([P, NKC, NH], bf16, tag="wz")
                for mc in range(NKC):
                    nc.gpsimd.tensor_scalar_mul(
                        wz[:, mc, :], maskT[:, mc, :], ksum[:, mc, si:si + 1])
                for mc in range(NKC):
                    nc.tensor.matmul(
                        zden_ps[0:NH, si, :],
                        wz[:, mc, :],
                        qe[:, mc, si * T:(si + 1) * T],
                        start=(mc == 0),
                        stop=(mc == NKC - 1),
                    )
            nc.vector.reciprocal_approx_fast(
                z_all[0:NH, s0:s0 + 2, :], zden_ps[0:NH, :, :])

        prev = None
        for p in range(S // 2):
            cur = emit_proj(p)
            if p == 0:
                for wT, wd in ((wvT, wvT_d), (wpT, wpT_d)):
                    nc.gpsimd.dma_start(
                        out=wT[:], in_=wd.rearrange("(a p) d -> p a d", p=P))
            if prev is not None:
                emit_tail(prev)
            prev = cur
        emit_tail(prev)

        # =================== phase B: u (batched over slices) ===============
        # uT[e, n, slice] = sum_c WvT[c, n*D+e] * sT[c, n, slice]
        ut_ps = ps_sm.tile([P, NKC, S], f32, tag="st")
        for n in range(NH):
            r0 = 64 * (n % 2)
            for kc in range(NKC):
                nc.tensor.matmul(
                    ut_ps[r0:r0 + 64, n // 2, :],
                    wvT[:, kc, n * D:(n + 1) * D],
                    sT_all[:, kc, :, n],
                    start=(kc == 0),
                    stop=(kc == NKC - 1),
                )
        nc.scalar.copy(uT_sb[:], ut_ps[:])

        # cast z to bf16 for the final matmuls (overlaps the GM build below)
        zbs = []
        for pr in range(S // 2):
            zb = zb_pool.tile([P, 2, T], bf16, tag="zb")
            nc.vector.tensor_copy(zb[0:NH, :, :], z_all[0:NH, 2 * pr:2 * pr + 2, :])
            zbs.append(zb)

        # ============ phase C: batched w, then per-slice out ================
        # GM_all[c, ci, 8*s + n] = maskT[c, ci, n] * uT[c, ci, s]
        GM_all = persist.tile([P, NKC, S * NH], bf16, tag="GM")
        engs = (nc.scalar, nc.gpsimd, nc.vector)
        idx = 0
        for ci in range(NKC):
            for s in range(S):
                eng = engs[idx % 3]
                idx += 1
                if eng is nc.scalar:
                    nc.scalar.mul(
                        GM_all[:, ci, 8 * s:8 * s + 8],
                        maskT[:, ci, :], uT_sb[:, ci, s:s + 1])
                else:
                    eng.tensor_scalar_mul(
                        GM_all[:, ci, 8 * s:8 * s + 8],
                        maskT[:, ci, :], uT_sb[:, ci, s:s + 1])

        # W_ps[8*s + n, cO] = sum_c GM_all[c, (8s+n)] * WpT[c, cO]
        w_ps = ps_proj.tile([P, C], f32, tag="proj")
        for ci in range(NKC):
            nc.tensor.matmul(
                w_ps[:],
                GM_all[:, ci, :],
                wpT[:, ci, :],
                start=(ci == 0),
                stop=(ci == NKC - 1),
            )
        w4sb = persist.tile([P, C], bf16, tag="w4sb")
        nc.scalar.copy(w4sb[:], w_ps[:])
        # shuffle rows (8s+n) -> partition n, free s via a DRAM round-trip
        # (same DGE queue => ordered)
        nc.scalar.dma_start(out=wtmp_d[:, :], in_=w4sb[:])
        w4stk = persist.tile([P, S, C], bf16, tag="w4stk")
        nc.scalar.dma_start(
            out=w4stk[0:NH, :, :],
            in_=wtmp_d.rearrange("(s n) c -> n s c", n=NH),
        )

        for s in range(S):
            osb = osb_pool.tile([P, NTC, C], f32, tag="outsb")
            for tcb in range(NTC):
                o_ps = ps_proj.tile([P, C], f32, tag="proj")
                nc.tensor.matmul(
                    o_ps[:],
                    zbs[s // 2][0:NH, s % 2, tcb * P:(tcb + 1) * P],
                    w4stk[0:NH, s, :],
                    start=True,
                    stop=True,
                )
                if tcb == 0:
                    nc.scalar.copy(osb[:, tcb, :], o_ps[:])
                else:
                    nc.vector.tensor_copy(osb[:, tcb, :], o_ps[:])
            nc.gpsimd.dma_start(
                out=out_d[s].rearrange("(a p) c -> p a c", p=P),
                in_=osb[:],
            )

    nc.compile()
    return nc


def _get_nc():
    if "nc" not in _BUILT:
        _BUILT["nc"] = _build_nc()
    return _BUILT["nc"]


def kernel(**inputs):
    import ml_dtypes

    bf16 = ml_dtypes.bfloat16
    x = np.asarray(inputs["x"], dtype=np.float32)
    Wq = np.asarray(inputs["Wq"], dtype=np.float32)
    Wk = np.asarray(inputs["Wk"], dtype=np.float32)
    Wv = np.asarray(inputs["Wv"], dtype=np.float32)
    Wp = np.asarray(inputs["Wp"], dtype=np.float32)
    bp = np.asarray(inputs.get("bp", np.zeros(C)), dtype=np.float32)

    x16 = np.ascontiguousarray(x.reshape(B * M, T, C).astype(bf16))
    xT16 = np.ascontiguousarray(x16.transpose(0, 2, 1))
    wqT16 = np.ascontiguousarray(Wq.T.astype(bf16))
    wkT16 = np.ascontiguousarray(Wk.T.astype(bf16))
    wvT16 = np.ascontiguousarray(Wv.T.astype(bf16))
    wpT16 = np.ascontiguousarray(Wp.T.astype(bf16))
    in_maps = []
    for i in range(NCORES):
        in_maps.append({
            "x16": np.ascontiguousarray(x16[S * i:S * (i + 1)]),
            "xT16": np.ascontiguousarray(xT16[S * i:S * (i + 1)]),
            "WqT16": wqT16, "WkT16": wkT16, "WvT16": wvT16, "WpT16": wpT16,
        })

    from concourse.bass_utils import run_bass_kernel_spmd

    nc = _get_nc()
    trace = os.environ.get("KERNEL_TRACE", "0") == "1"
    tdir = os.environ.get("KERNEL_TRACE_DIR") or None
    res = run_bass_kernel_spmd(nc, in_maps, list(range(NCORES)), trace=trace,
                               tmpdir=tdir)
    if trace and res.exec_time_ns is not None:
        print(f"HW exec time: {res.exec_time_ns} ns", flush=True)
        _BUILT["exec_time_ns"] = res.exec_time_ns
    if trace and res.instructions_and_trace is not None:
        _BUILT["trace_path"] = res.instructions_and_trace[1]

    out = np.concatenate([res.results[i]["out"] for i in range(NCORES)], axis=0)
    out = out.reshape(B, M, T, C)
    if np.any(bp):
        out = out + bp
    return out.astype(np.float32)


# revision 10
# speedup vs baseline: 1.2367x; 1.1017x over previous
"""Trainium2 Bass kernel for nn_KernelAxialMultiAttention (linear attention).

Math (per independent (b, m) slice; x: [T=256, C=512], N=8 heads, D=64):
  q = elu(x @ Wq.T) + 1          [T, C]   (heads along C)
  k = elu(x @ Wk.T) + 1
  ksum[c]   = sum_t k[t, c]
  krow[n,t] = sum_{c in head n} k[t, c]
  zden[n,t] = sum_{c in head n} q[t, c] * ksum[c];  z = 1/zden
  s[n, c]   = sum_t krow[n, t] * x[t, c]
  u[n, e]   = sum_c s[n, c] * Wv[n*D+e, c]     (= KtV column sums)
  w[n, cO]  = sum_e u[n, e] * Wp[cO, n*D+e]
  out[t,cO] = sum_n z[n, t] * w[n, cO]
Algebraically identical to the reference (sum reordering only); the
v-projection and output projection collapse because Z is constant over D.

elu(x)+1 = max(x,0) + exp(min(x,0)); computed per PSUM tile either as
  Q-variant: Scalar relu(-x), Scalar exp(-r), DVE (max(x,0) + e)
  K-variant: Scalar exp(x), DVE min(e,1), DVE (max(x,0) + c)
to balance Scalar vs DVE load.

All TensorEngine operands are bf16 (fp32 moving operand runs at 1/4 rate);
PSUM accumulation is fp32 everywhere.  The per-pair loop is software
pipelined: the projection matmuls of pair p are emitted before the small
reduction matmuls of pair p-1 so the PE never starves.

Sharding: data-parallel over the 128 (b, m) slices -> 16 per NeuronCore.
"""

import os
import sys

import numpy as np

for _p in ("/opt/trn_rl_repo", "/root/.axon_site/_ro/trn_rl_repo"):
    if os.path.isdir(_p) and _p not in sys.path:
        sys.path.insert(0, _p)

B, M, T, C = 2, 64, 256, 512
NH, D = 8, 64
S = 16            # slices per core
NCORES = 8
P = 128           # partitions
NKC = C // P      # 4 contraction chunks
NTC = T // P      # 2 t chunks

_BUILT = {}


def _register_elu_op():
    """Register a fused custom-DVE op: out = max(in0, 0) + min(in1, s0).

    This is the documented extension point for custom DVE ops
    (concourse/dve_ops.py docstring); we register at runtime since the
    repo tree is read-only here."""
    import concourse.dve_ops as dve_ops
    for op in dve_ops.OPS:
        if op.name == "ELU1_COMBINE_ANT":
            return op
    from concourse.dve_spec import (
        C0, Spec, Src0, Src1, Zero, _has_src1, lower, maxx, minn,
    )
    from concourse.dve_uop import DveOpSpec

    name = "ELU1_COMBINE_ANT"
    row = dve_ops._CUSTOM_DVE_ROW_BASE + len(dve_ops.OPS)
    assert row < 0x20
    dve_ops._SUB_OPCODE_FOR_NAME[name] = row
    spec = Spec(
        body=maxx(Src0, Zero) + minn(Src1, C0),
        reference=lambda in0, in1, s0, s1, imm2: (
            np.maximum(in0, 0.0) + np.minimum(in1, s0)
        ).astype(np.float32),
    )
    shas = {}
    for ver in ("v3", "v4"):
        try:
            uops = lower(spec, ver=ver)
            shas[ver] = DveOpSpec(
                name=name, opcode=row, uops=uops, rd1_en=_has_src1(spec)
            ).sha(ver)
        except Exception:
            pass
    op = dve_ops.DveOp(name, spec, subdim=False, uops_sha=shas)
    dve_ops.OPS.append(op)
    dve_ops.CUSTOM_DVE_SPECS[name] = spec
    return op


def _build_nc():
    from contextlib import ExitStack

    import concourse.bacc as bacc
    import concourse.bass as bass
    import concourse.mybir as mybir
    import concourse.tile as tile
    from concourse.masks import make_identity

    f32 = mybir.dt.float32
    bf16 = mybir.dt.bfloat16
    AF = mybir.ActivationFunctionType
    OP = mybir.AluOpType
    AX = mybir.AxisListType

    elu_op = _register_elu_op()

    nc = bacc.Bacc(None, target_bir_lowering=False)
    x_d = nc.declare_dram_parameter("x16", [S, T, C], bf16, isOutput=False)
    xT_d = nc.declare_dram_parameter("xT16", [S, C, T], bf16, isOutput=False)
    wqT_d = nc.declare_dram_parameter("WqT16", [C, C], bf16, isOutput=False)
    wkT_d = nc.declare_dram_parameter("WkT16", [C, C], bf16, isOutput=False)
    wvT_d = nc.declare_dram_parameter("WvT16", [C, C], bf16, isOutput=False)
    wpT_d = nc.declare_dram_parameter("WpT16", [C, C], bf16, isOutput=False)
    out_d = nc.declare_dram_parameter("out", [S, T, C], f32, isOutput=True)
    wtmp_d = nc.declare_dram_parameter("wtmp", [S * NH, C], bf16, isOutput=True)

    with tile.TileContext(nc) as tc, ExitStack() as ctx:
        wpool = ctx.enter_context(tc.tile_pool(name="weights", bufs=1))
        cpool = ctx.enter_context(tc.tile_pool(name="consts", bufs=1))
        persist = ctx.enter_context(tc.tile_pool(name="persist", bufs=1))
        xn_pool = ctx.enter_context(tc.tile_pool(name="xnat", bufs=4))
        xt_pool = ctx.enter_context(tc.tile_pool(name="xT", bufs=2))
        ex_pool = ctx.enter_context(tc.tile_pool(name="expt", bufs=3))
        qe_pool = ctx.enter_context(tc.tile_pool(name="qe", bufs=2))
        ke_pool = ctx.enter_context(tc.tile_pool(name="ke", bufs=2))
        ksum_pool = ctx.enter_context(tc.tile_pool(name="ksum", bufs=2))
        krow_pool = ctx.enter_context(tc.tile_pool(name="krow", bufs=2))
        krt_pool = ctx.enter_context(tc.tile_pool(name="krowT", bufs=2))
        wz_pool = ctx.enter_context(tc.tile_pool(name="wz", bufs=2))
        zb_pool = ctx.enter_context(tc.tile_pool(name="zb", bufs=8))
        w4sb_pool = ctx.enter_context(tc.tile_pool(name="w4sb", bufs=2))
        osb_pool = ctx.enter_context(tc.tile_pool(name="outsb", bufs=3))

        ps_proj = ctx.enter_context(
            tc.tile_pool(name="ps_proj", bufs=4, space=bass.MemorySpace.PSUM))
        ps_z = ctx.enter_context(
            tc.tile_pool(name="ps_z", bufs=1, space=bass.MemorySpace.PSUM))
        ps_tr = ctx.enter_context(
            tc.tile_pool(name="ps_tr", bufs=1, space=bass.MemorySpace.PSUM))
        ps_sm = ctx.enter_context(
            tc.tile_pool(name="ps_sm", bufs=1, space=bass.MemorySpace.PSUM))
        ps_kr = ctx.enter_context(
            tc.tile_pool(name="ps_kr", bufs=1, space=bass.MemorySpace.PSUM))

        # ---- weights (host-pretransposed) into SBUF ----
        # layout [c % 128, c // 128, row]
        wqT = wpool.tile([P, NKC, C], bf16, tag="wqT")
        wkT = wpool.tile([P, NKC, C], bf16, tag="wkT")
        wvT = wpool.tile([P, NKC, C], bf16, tag="wvT")
        wpT = wpool.tile([P, NKC, C], bf16, tag="wpT")
        for wT, wd in ((wqT, wqT_d), (wkT, wkT_d)):
            nc.gpsimd.dma_start(
                out=wT[:], in_=wd.rearrange("(a p) d -> p a d", p=P))

        # ---- head-block masks: maskT[:, ci, n] = 1 if (128*ci + p)//64 == n ----
        maskT = cpool.tile([P, NKC, NH], bf16, tag="maskT")
        nc.gpsimd.memset(maskT[:], 0.0)
        for ci in range(NKC):
            nc.gpsimd.memset(maskT[0:64, ci, 2 * ci:2 * ci + 1], 1.0)
            nc.gpsimd.memset(maskT[64:128, ci, 2 * ci + 1:2 * ci + 2], 1.0)
        ident = cpool.tile([P, P], bf16, tag="ident")
        make_identity(nc, ident[:])

        sT_all = persist.tile([P, NKC, S, NH], bf16, tag="sT_all")
        z_all = persist.tile([P, S, T], f32, tag="z_all")
        uT_sb = persist.tile([P, NKC, S], f32, tag="uT_sb")

        w4stk = persist.tile([P, S, C], bf16, tag="w4stk")
        x3 = x_d  # [S, T, C] bf16

        # ---------------- phase A helpers (software pipelined) --------------
        def emit_proj(p):
            s0, s1 = 2 * p, 2 * p + 1
            xT = xt_pool.tile([P, NKC, 2, T], bf16, tag="xT")
            for si, s in ((0, s0), (1, s1)):
                nc.sync.dma_start(
                    out=xT[:, :, si, :],
                    in_=xT_d[s].rearrange("(a p) t -> p a t", p=P),
                )
            xn = []
            for s in (s0, s1):
                t_ = xn_pool.tile([P, NTC, C], bf16, tag="xnat")
                nc.gpsimd.dma_start(
                    out=t_[:],
                    in_=x3[s].rearrange("(a p) c -> p a c", p=P),
                )
                xn.append(t_)

            ksum = ksum_pool.tile([P, NKC, 2], f32, tag="ksum")
            qe = qe_pool.tile([P, NKC, 2 * T], bf16, tag="qe")
            ke = ke_pool.tile([P, NKC, 2 * T], bf16, tag="ke")
            for wT, etile, is_k in ((wqT, qe, False), (wkT, ke, True)):
                for mc in range(NKC):
                    pp = ps_proj.tile([P, 2 * T], f32, tag="proj")
                    for kc in range(NKC):
                        nc.tensor.matmul(
                            pp[:],
                            wT[:, kc, mc * P:(mc + 1) * P],
                            xT[:, kc, :, :],
                            start=(kc == 0),
                            stop=(kc == NKC - 1),
                        )
                    # elu(x)+1 = max(x,0) + min(exp(x),1): Scalar exp,
                    # then one fused custom-DVE combine.
                    ex = ex_pool.tile([P, 2 * T], bf16, tag="expt")
                    nc.scalar.activation(ex[:], pp[:], AF.Exp)
                    nc.vector._custom_dve(
                        elu_op, out=etile[:, mc, :], in0=pp[:], in1=ex[:],
                        s0=1.0)
                    if is_k:
                        nc.vector.tensor_reduce(
                            ksum[:, mc, :],
                            etile[:, mc, :].rearrange("p (a t) -> p a t", a=2),
                            AX.X, OP.add)
            return dict(p=p, s0=s0, s1=s1, xn=xn, qe=qe, ke=ke, ksum=ksum)

        def emit_tail(st):
            s0, s1, xn = st["s0"], st["s1"], st["xn"]
            qe, ke, ksum = st["qe"], st["ke"], st["ksum"]
            # krow[n, t2] = sum_c maskT[c, n] * ke[c, t2]   (t2 covers both slices)
            krow_ps = ps_kr.tile([P, 2 * T], f32, tag="krow")
            for mc in range(NKC):
                nc.tensor.matmul(
                    krow_ps[0:NH, :],
                    maskT[:, mc, :],
                    ke[:, mc, :],
                    start=(mc == 0),
                    stop=(mc == NKC - 1),
                )
            krow_sb = krow_pool.tile([P, 2 * T], bf16, tag="krow")
            nc.scalar.copy(krow_sb[0:NH, :], krow_ps[0:NH, :])
            # transpose 128-col chunks: krt[t, j, n], j = 2*si + tcb
            krt_tr = ps_tr.tile([P, NKC, NH], bf16, tag="ktr")
            for j in range(4):
                nc.tensor.transpose(
                    krt_tr[:, j, :],
                    krow_sb[0:NH, j * P:(j + 1) * P],
                    ident[0:NH, 0:NH],
                )
            krt = krt_pool.tile([P, NKC, NH], bf16, tag="krt")
            nc.vector.tensor_copy(krt[:], krt_tr[:])

            zden_ps = ps_z.tile([P, 2, T], f32, tag="zden")
            for si, s in ((0, s0), (1, s1)):
                # sT[c, n] = sum_t x[t, c] * krowT[t, n]
                st_ps = ps_sm.tile([P, NKC, NH], f32, tag="st")
                for mc in range(NKC):
                    for tcb in range(NTC):
                        nc.tensor.matmul(
                            st_ps[:, mc, :],
                            xn[si][:, tcb, mc * P:(mc + 1) * P],
                            krt[:, 2 * si + tcb, :],
                            start=(tcb == 0),
                            stop=(tcb == NTC - 1),
                        )
                nc.vector.tensor_copy(sT_all[:, :, s, :], st_ps[:])

                # zden[n, t] = sum_c (maskT*ksum)[c, n] * qe[c, t]
                wz = wz_pool.tile([P, NKC, NH], bf16, tag="wz")
                for mc in range(NKC):
                    nc.gpsimd.tensor_scalar_mul(
                        wz[:, mc, :], maskT[:, mc, :], ksum[:, mc, si:si + 1])
                for mc in range(NKC):
                    nc.tensor.matmul(
                        zden_ps[0:NH, si, :],
                        wz[:, mc, :],
                        qe[:, mc, si * T:(si + 1) * T],
                        start=(mc == 0),
                        stop=(mc == NKC - 1),
                    )
            nc.vector.reciprocal_approx_fast(
                z_all[0:NH, s0:s0 + 2, :], zden_ps[0:NH, :, :])

        # ---------- phase B/C emitted in halves, interleaved into phase A ----
        zbs = [None] * (S // 2)

        def emit_zb(pr):
            zb = zb_pool.tile([P, 2, T], bf16, tag="zb")
            nc.vector.tensor_copy(zb[0:NH, :, :], z_all[0:NH, 2 * pr:2 * pr + 2, :])
            zbs[pr] = zb

        engs = (nc.scalar, nc.gpsimd, nc.vector)

        def emit_u_gm_half(h):
            # uT[e, n, s] = sum_c WvT[c, n*D+e] * sT[c, n, s] for 8 slices
            ut_ps = ps_sm.tile([P, NKC, NH], f32, tag="st")
            for n in range(NH):
                r0 = 64 * (n % 2)
                for kc in range(NKC):
                    nc.tensor.matmul(
                        ut_ps[r0:r0 + 64, n // 2, :],
                        wvT[:, kc, n * D:(n + 1) * D],
                        sT_all[:, kc, 8 * h:8 * h + 8, n],
                        start=(kc == 0),
                        stop=(kc == NKC - 1),
                    )
            nc.scalar.copy(uT_sb[:, :, 8 * h:8 * h + 8], ut_ps[:])
            # GM_all[c, ci, 8*s + n] = maskT[c, ci, n] * uT[c, ci, s]
            idx = 0
            for ci in range(NKC):
                for j in range(8):
                    s = 8 * h + j
                    eng = engs[idx % 3]
                    idx += 1
                    if eng is nc.scalar:
                        nc.scalar.mul(
                            GM_all[:, ci, 8 * s:8 * s + 8],
                            maskT[:, ci, :], uT_sb[:, ci, s:s + 1])
                    else:
                        eng.tensor_scalar_mul(
                            GM_all[:, ci, 8 * s:8 * s + 8],
                            maskT[:, ci, :], uT_sb[:, ci, s:s + 1])

        def emit_w_half(h):
            # W[8s+n, cO] = sum_c GM_all[c, 8s+n] * WpT[c, cO]  (64 rows)
            w_ps = ps_proj.tile([P, C], f32, tag="proj")
            for ci in range(NKC):
                nc.tensor.matmul(
                    w_ps[0:64, :],
                    GM_all[:, ci, 64 * h:64 * h + 64],
                    wpT[:, ci, :],
                    start=(ci == 0),
                    stop=(ci == NKC - 1),
                )
            w4sb = w4sb_pool.tile([P, C], bf16, tag="w4sb")
            nc.scalar.copy(w4sb[0:64, :], w_ps[0:64, :])
            # shuffle rows (8s+n) -> partition n, free s via a DRAM round-trip
            # (same DGE queue => ordered)
            nc.scalar.dma_start(
                out=wtmp_d[64 * h:64 * h + 64, :], in_=w4sb[0:64, :])
            nc.scalar.dma_start(
                out=w4stk[0:NH, 8 * h:8 * h + 8, :],
                in_=wtmp_d.rearrange("(s n) c -> n s c", n=NH)[
                    :, 8 * h:8 * h + 8, :],
            )

        def emit_out(slices):
            for s in slices:
                osb = osb_pool.tile([P, NTC, C], f32, tag="outsb")
                for tcb in range(NTC):
                    o_ps = ps_proj.tile([P, C], f32, tag="proj")
                    nc.tensor.matmul(
                        o_ps[:],
                        zbs[s // 2][0:NH, s % 2, tcb * P:(tcb + 1) * P],
                        w4stk[0:NH, s, :],
                        start=True,
                        stop=True,
                    )
                    if tcb == 0:
                        nc.scalar.copy(osb[:, tcb, :], o_ps[:])
                    else:
                        nc.vector.tensor_copy(osb[:, tcb, :], o_ps[:])
                nc.gpsimd.dma_start(
                    out=out_d[s].rearrange("(a p) c -> p a c", p=P),
                    in_=osb[:],
                )

        GM_all = persist.tile([P, NKC, S * NH], bf16, tag="GM")

        prev = None
        for p in range(S // 2):
            cur = emit_proj(p)
            if p == 0:
                for wT, wd in ((wvT, wvT_d), (wpT, wpT_d)):
                    nc.gpsimd.dma_start(
                        out=wT[:], in_=wd.rearrange("(a p) d -> p a d", p=P))
            if prev is not None:
                emit_tail(prev)
            if p == 5:
                emit_u_gm_half(0)
            elif p == 6:
                emit_w_half(0)
                for pr in range(4):
                    emit_zb(pr)
            elif p == 7:
                emit_out(range(0, 6))
            prev = cur
        emit_tail(prev)
        emit_u_gm_half(1)
        for pr in range(4, 8):
            emit_zb(pr)
        emit_w_half(1)
        emit_out(range(6, S))

    nc.compile()
    return nc


def _get_nc():
    if "nc" not in _BUILT:
        _BUILT["nc"] = _build_nc()
    return _BUILT["nc"]


def kernel(**inputs):
    import ml_dtypes

    bf16 = ml_dtypes.bfloat16
    x = np.asarray(inputs["x"], dtype=np.float32)
    Wq = np.asarray(inputs["Wq"], dtype=np.float32)
    Wk = np.asarray(inputs["Wk"], dtype=np.float32)
    Wv = np.asarray(inputs["Wv"], dtype=np.float32)
    Wp = np.asarray(inputs["Wp"], dtype=np.float32)
    bp = np.asarray(inputs.get("bp", np.zeros(C)), dtype=np.float32)

    x16 = np.ascontiguousarray(x.reshape(B * M, T, C).astype(bf16))
    xT16 = np.ascontiguousarray(x16.transpose(0, 2, 1))
    wqT16 = np.ascontiguousarray(Wq.T.astype(bf16))
    wkT16 = np.ascontiguousarray(Wk.T.astype(bf16))
    wvT16 = np.ascontiguousarray(Wv.T.astype(bf16))
    wpT16 = np.ascontiguousarray(Wp.T.astype(bf16))
    in_maps = []
    for i in range(NCORES):
        in_maps.append({
            "x16": np.ascontiguousarray(x16[S * i:S * (i + 1)]),
            "xT16": np.ascontiguousarray(xT16[S * i:S * (i + 1)]),
            "WqT16": wqT16, "WkT16": wkT16, "WvT16": wvT16, "WpT16": wpT16,
        })

    from concourse.bass_utils import run_bass_kernel_spmd

    nc = _get_nc()
    trace = os.environ.get("KERNEL_TRACE", "0") == "1"
    tdir = os.environ.get("KERNEL_TRACE_DIR") or None
    res = run_bass_kernel_spmd(nc, in_maps, list(range(NCORES)), trace=trace,
                               tmpdir=tdir)
    if trace and res.exec_time_ns is not None:
        print(f"HW exec time: {res.exec_time_ns} ns", flush=True)
        _BUILT["exec_time_ns"] = res.exec_time_ns
    if trace and res.instructions_and_trace is not None:
        _BUILT["trace_path"] = res.instructions_and_trace[1]

    out = np.concatenate([res.results[i]["out"] for i in range(NCORES)], axis=0)
    out = out.reshape(B, M, T, C)
    if np.any(bp):
        out = out + bp
    return out.astype(np.float32)
